# revision 9
# baseline (speedup 1.0000x reference)
"""GNN message-passing (GCN-mean) kernel for 8 Trainium2 NeuronCores, v2.

Structure vs v1:
  - Variable chunk counts per (block, src-group): edges packed tight, pad
    slots gather row 0 with dstrel=-1 (one-hot kills them).
  - Supergathers: one dma_gather per (supergroup of K blocks, src-group)
    amortizes the ~1.9us SWDGE descriptor-gen cost on the Pool engine.
  - One-hot built per block in ONE big DVE op over [P, S_b, P] instead of
    S_b small tensor_scalar ops (kills the 10ms DVE bottleneck).
  - Static num_idxs (no cnt loads / reg_loads).
"""
import math
import numpy as np

NC = 8
P = 128
EPS = 1e-5
K_SG = 6          # blocks per supergather
ONE_HOT = "tt_bcast"   # "tt_bcast" | "ts_chunk"
SINGLE_PACKET = False

_CACHE = {}
_LAST_EXEC = None


# --------------------------------------------------------------------------
# host-side edge packing
# --------------------------------------------------------------------------
def _prep_edges(src, dst, N, SH, SHP, NB):
    """Returns per-core packing:
      idx_flat   [NC][P, ICOLS]  int16  wrapped gather indices
      dst_flat   [NC][P, DCOLS]  bf16-able f32 dstrel slabs (per block contiguous)
      layout     [NC]: list over sg of dict(
                   gcols: list of (col_off, ncols, cap) per g (cap=0 -> skip),
                   blocks: list of (b, dcol_off, S_b, pos_list) )
    """
    E = src.shape[0]
    src = src.astype(np.int64)
    dst = dst.astype(np.int64)
    core = dst // SH
    dst_loc = dst - core * SH
    blk = dst_loc >> 7
    dst_rel = dst_loc & 127
    grp = src // SH
    src_loc = src - grp * SH

    key = ((core * NB + blk) * NC + grp)
    order = np.argsort(key * N + src, kind="stable")
    ks = key[order]
    sl = src_loc[order].astype(np.int16)
    dr = dst_rel[order].astype(np.float32)
    counts = np.bincount(ks, minlength=NC * NB * NC).reshape(NC, NB, NC)
    starts = np.zeros(NC * NB * NC, np.int64)
    np.cumsum(counts.reshape(-1)[:-1], out=starts[1:])
    starts = starts.reshape(NC, NB, NC)

    NSG = (NB + K_SG - 1) // K_SG
    idx_flat, dst_flat, layouts = [], [], []
    for c in range(NC):
        idx_cols, dst_cols, sgs = [], [], []
        icol = 0
        dcol = 0
        for sgi in range(NSG):
            bs = range(sgi * K_SG, min((sgi + 1) * K_SG, NB))
            gcols = []
            # chunk position enumeration: g-major, then block, then chunk
            pos = 0
            pos_of = {}
            for g in range(NC):
                cap = 0
                seg = []
                for b in bs:
                    cnt = counts[c, b, g]
                    ch = -(-cnt // 128) if cnt else 0
                    if ch:
                        s0 = starts[c, b, g]
                        a = np.zeros(ch * 128, np.int16)
                        a[:cnt] = sl[s0:s0 + cnt]
                        seg.append(a)
                        pos_of[(b, g)] = (pos, ch, cnt)
                        pos += ch
                        cap += ch * 128
                if cap:
                    arr = np.concatenate(seg)
                    S8 = cap // 16
                    w = np.ascontiguousarray(
                        np.tile(arr.reshape(S8, 16).T, (8, 1)))
                    idx_cols.append(w)
                    gcols.append((icol, S8, cap))
                    icol += S8
                else:
                    gcols.append((icol, 0, 0))
            blocks = []
            for b in bs:
                # block b's chunks in (g asc, chunk asc) order
                pos_list = []
                dslab = []
                for g in range(NC):
                    if (b, g) not in pos_of:
                        continue
                    p0, ch, cnt = pos_of[(b, g)]
                    s0 = starts[c, b, g]
                    d = np.full(ch * 128, -1.0, np.float32)
                    d[:cnt] = dr[s0:s0 + cnt]
                    dslab.append(d.reshape(ch, 128).T)  # [128, ch]
                    pos_list.extend(range(p0, p0 + ch))
                S_b = len(pos_list)
                if S_b:
                    dst_cols.append(np.concatenate(dslab, axis=1))
                blocks.append((b, dcol, S_b, pos_list))
                dcol += S_b
            sgs.append({"gcols": gcols, "blocks": blocks, "sgc": pos})
        idx_flat.append(np.concatenate(idx_cols, axis=1) if idx_cols
                        else np.zeros((P, 1), np.int16))
        dst_flat.append(np.concatenate(dst_cols, axis=1) if dst_cols
                        else np.zeros((P, 1), np.float32))
        layouts.append(sgs)
    # pad all cores to same shapes (SPMD: one program, shapes must match)
    icmax = max(a.shape[1] for a in idx_flat)
    dcmax = max(a.shape[1] for a in dst_flat)
    idx_flat = [np.pad(a, ((0, 0), (0, icmax - a.shape[1]))) for a in idx_flat]
    dst_flat = [np.pad(a, ((0, 0), (0, dcmax - a.shape[1])),
                       constant_values=-1.0) for a in dst_flat]
    return idx_flat, dst_flat, layouts


def _layout_signature(layouts):
    """Hashable program-shape signature (per-core layouts must be merged:
    SPMD runs ONE program, so per-sg slices must be identical across cores.
    We take the max shape per slot and pad; pos_list/caps differ per core...
    -> instead: build the program per-core-0 layout? No: program must be
    valid for all cores. We make the LAYOUT identical across cores by
    padding counts to the max across cores during prep."""
    raise NotImplementedError


# --------------------------------------------------------------------------
# NOTE: SPMD constraint - one program for all cores. So chunk layout must be
# IDENTICAL across cores. We therefore compute counts as max over cores per
# (b, g) and pad every core's edge list to that count.
# --------------------------------------------------------------------------
def _prep_edges_uniform(src, dst, N, SH, SHP, NB):
    E = src.shape[0]
    src = src.astype(np.int64)
    dst = dst.astype(np.int64)
    core = dst // SH
    dst_loc = dst - core * SH
    blk = dst_loc >> 7
    dst_rel = dst_loc & 127
    grp = src // SH
    src_loc = src - grp * SH

    key = ((core * NB + blk) * NC + grp)
    order = np.argsort(key * N + src, kind="stable")
    ks = key[order]
    sl = src_loc[order].astype(np.int16)
    dr = dst_rel[order].astype(np.float32)
    counts = np.bincount(ks, minlength=NC * NB * NC).reshape(NC, NB, NC)
    starts = np.zeros(NC * NB * NC, np.int64)
    np.cumsum(counts.reshape(-1)[:-1], out=starts[1:])
    starts = starts.reshape(NC, NB, NC)

    chunks = -(-np.max(counts, axis=0) // 128)   # [NB, NC] max over cores
    NSG = (NB + K_SG - 1) // K_SG

    sgs = []
    icol_tot = 0
    dcol_tot = 0
    for sgi in range(NSG):
        bs = list(range(sgi * K_SG, min((sgi + 1) * K_SG, NB)))
        gcols = []
        pos = 0
        pos_of = {}
        icol = icol_tot
        for g in range(NC):
            cap = 0
            for b in bs:
                ch = chunks[b, g]
                if ch:
                    pos_of[(b, g)] = (pos, ch)
                    pos += ch
                    cap += ch * 128
            S8 = cap // 16
            gcols.append((icol, S8, cap))
            icol += S8
        blocks = []
        dcol = dcol_tot
        for b in bs:
            pos_list = []
            for g in range(NC):
                if (b, g) in pos_of:
                    p0, ch = pos_of[(b, g)]
                    pos_list.extend(range(p0, p0 + ch))
            S_b = len(pos_list)
            blocks.append((b, dcol, S_b, pos_list))
            dcol += S_b
        sgs.append({"gcols": gcols, "blocks": blocks, "sgc": pos,
                    "icol0": icol_tot, "icols": icol - icol_tot,
                    "dcol0": dcol_tot, "dcols": dcol - dcol_tot})
        icol_tot = icol
        dcol_tot = dcol

    # fill per-core data
    idx_flat = np.zeros((NC, P, icol_tot), np.int16)
    dst_flat = np.full((NC, P, dcol_tot), -1.0, np.float32)
    for c in range(NC):
        for sg in sgs:
            for g in range(NC):
                icol, S8, cap = sg["gcols"][g]
                if not cap:
                    continue
                arr = np.zeros(cap, np.int16)
                off = 0
                for (b, dcol, S_b, _pl) in sg["blocks"]:
                    ch = chunks[b, g]
                    if not ch:
                        continue
                    cnt = counts[c, b, g]
                    s0 = starts[c, b, g]
                    arr[off:off + cnt] = sl[s0:s0 + cnt]
                    off += ch * 128
                w = np.tile(arr.reshape(S8, 16).T, (8, 1))
                idx_flat[c, :, icol:icol + S8] = w
            for (b, dcol, S_b, _pl) in sg["blocks"]:
                dj = 0
                for g in range(NC):
                    ch = chunks[b, g]
                    if not ch:
                        continue
                    cnt = counts[c, b, g]
                    s0 = starts[c, b, g]
                    d = np.full(ch * 128, -1.0, np.float32)
                    d[:cnt] = dr[s0:s0 + cnt]
                    dst_flat[c, :, dcol + dj:dcol + dj + ch] = \
                        d.reshape(ch, 128).T
                    dj += ch
    return idx_flat, dst_flat, sgs


# --------------------------------------------------------------------------
# device program
# --------------------------------------------------------------------------
def _build_program(NB, SHP, sgs, ICOLS, DCOLS, LAYERS, OUT):
    import concourse.bacc as bacc
    import concourse.mybir as mybir
    import concourse.tile as tile
    from concourse.masks import make_identity

    f32 = mybir.dt.float32
    bf16 = mybir.dt.bfloat16
    i16 = mybir.dt.int16
    Alu = mybir.AluOpType
    Act = mybir.ActivationFunctionType

    SGC_MAX = max(sg["sgc"] for sg in sgs)
    SB_MAX = max(S_b for sg in sgs for (_b, _d, S_b, _p) in sg["blocks"])
    IC_MAX = max(sg["icols"] for sg in sgs)
    DC_MAX = max(sg["dcols"] for sg in sgs)

    nc = bacc.Bacc("TRN2", target_bir_lowering=False, num_devices=NC,
                   num_swdge_queues=4)

    nodes_own = nc.declare_dram_parameter("nodes_own", [SHP, P], f32, isOutput=False)
    idxs = nc.declare_dram_parameter("idxs", [P, ICOLS], i16, isOutput=False)
    dsts = nc.declare_dram_parameter("dsts", [P, DCOLS], bf16, isOutput=False)
    W_in = nc.declare_dram_parameter("W_in", [P, P], f32, isOutput=False)
    Wl = nc.declare_dram_parameter("Wl", [LAYERS, P, P], f32, isOutput=False)
    Wout = nc.declare_dram_parameter("Wout", [P, OUT], f32, isOutput=False)
    b_in_bc = nc.declare_dram_parameter("b_in_bc", [P, P], f32, isOutput=False)
    bl_bc = nc.declare_dram_parameter("bl_bc", [LAYERS, P, P], f32, isOutput=False)
    bout_bc = nc.declare_dram_parameter("bout_bc", [P, OUT], f32, isOutput=False)
    iota_big_d = nc.declare_dram_parameter("iota_big", [P, SB_MAX, P], bf16, isOutput=False)
    out_own = nc.declare_dram_parameter("out_own", [SHP, OUT], f32, isOutput=True)

    rg = [list(range(NC))]

    with tile.TileContext(nc) as tc:
        with (
            tc.tile_pool(name="const", bufs=1) as cpool,
            tc.tile_pool(name="dram", bufs=1, space="DRAM") as dpool,
            tc.tile_pool(name="work", bufs=3) as wpool,
            tc.tile_pool(name="ohp", bufs=3) as ohpool,
            tc.tile_pool(name="xgp", bufs=2) as xgpool,
            tc.tile_pool(name="psum", bufs=2, space="PSUM") as ppool,
        ):
            iota_big_t = cpool.tile([P, SB_MAX, P], bf16)
            nc.sync.dma_start(out=iota_big_t[:], in_=iota_big_d[:])
            ident = cpool.tile([P, P], f32)
            make_identity(nc, ident[:])
            ones_bf = cpool.tile([P, 1], bf16)
            nc.vector.memset(ones_bf[:], 1.0)
            eps_t = cpool.tile([P, 1], f32)
            nc.vector.memset(eps_t[:], EPS)
            W_in_t = cpool.tile([P, P], f32)
            nc.sync.dma_start(out=W_in_t[:], in_=W_in[:])
            Wout_t = cpool.tile([P, OUT], f32)
            nc.sync.dma_start(out=Wout_t[:], in_=Wout[:])
            bin_t = cpool.tile([P, P], f32)
            nc.sync.dma_start(out=bin_t[:], in_=b_in_bc[:])
            bl_t = []
            for l in range(LAYERS):
                t = cpool.tile([P, P], f32, name=f"bl{l}")
                nc.sync.dma_start(out=t[:], in_=bl_bc[l])
                bl_t.append(t)
            Wl_ts = []
            for l in range(LAYERS):
                t = cpool.tile([P, P], f32, name=f"wl{l}")
                nc.sync.dma_start(out=t[:], in_=Wl[l])
                Wl_ts.append(t)
            bout_t = cpool.tile([P, OUT], f32)
            nc.sync.dma_start(out=bout_t[:], in_=bout_bc[:])
            inv_t = cpool.tile([P, NB], f32)

            ag_in = [dpool.tile([SHP, P], bf16, name=f"ag_in{l}") for l in range(LAYERS)]
            x_full = [
                dpool.tile([NC * SHP, P], bf16, addr_space="Shared", name=f"x_full{l}")
                for l in range(LAYERS)
            ]

            # ---------------- Phase A: x0 = nodes @ W_in + b_in ----------
            with nc.named_scope("phaseA"):
                for b in range(NB):
                    nb_t = wpool.tile([P, P], f32, tag="nb")
                    nc.sync.dma_start(out=nb_t[:], in_=nodes_own[b * P:(b + 1) * P, :])
                    pT = ppool.tile([P, P], f32, tag="mT", space="PSUM")
                    nc.tensor.transpose(pT[:], nb_t[:], ident[:])
                    nT = wpool.tile([P, P], f32, tag="mTs")
                    nc.scalar.copy(out=nT[:], in_=pT[:])
                    ph = ppool.tile([P, P], f32, tag="h", space="PSUM")
                    nc.tensor.matmul(ph[:], lhsT=nT[:], rhs=W_in_t[:], start=True, stop=True)
                    x0b = wpool.tile([P, P], bf16, tag="xnext")
                    nc.vector.tensor_tensor(out=x0b[:], in0=ph[:], in1=bin_t[:], op=Alu.add)
                    nc.sync.dma_start(out=ag_in[0][b * P:(b + 1) * P, :], in_=x0b[:])
                nc.gpsimd.collective_compute(
                    "AllGather", Alu.bypass, replica_groups=rg,
                    ins=[ag_in[0][:].opt()], outs=[x_full[0][:].opt()],
                )

            # ---------------- Layers ------------------------------------
            for l in range(LAYERS):
                xf = x_full[l]
                last = l == LAYERS - 1
                with nc.named_scope(f"layer{l}"):
                    for sg in sgs:
                        idx_t = wpool.tile([P, IC_MAX], i16, tag="idx")
                        nc.sync.dma_start(
                            out=idx_t[:, :sg["icols"]],
                            in_=idxs[:, sg["icol0"]:sg["icol0"] + sg["icols"]])
                        dst_t = wpool.tile([P, DC_MAX], bf16, tag="dstb")
                        nc.sync.dma_start(
                            out=dst_t[:, :sg["dcols"]],
                            in_=dsts[:, sg["dcol0"]:sg["dcol0"] + sg["dcols"]])
                        xg = xgpool.tile([P, SGC_MAX, P], bf16, tag="xg")
                        pos0 = 0
                        for g in range(NC):
                            icol, S8, cap = sg["gcols"][g]
                            if not cap:
                                continue
                            nch = cap // 128
                            nc.gpsimd.dma_gather(
                                out_ap=xg[:, pos0:pos0 + nch, :],
                                in_ap=xf[g * SHP:(g + 1) * SHP, :],
                                idxs_ap=idx_t[:, icol - sg["icol0"]:
                                              icol - sg["icol0"] + S8],
                                num_idxs=cap,
                                num_idxs_reg=cap,
                                elem_size=P,
                                single_packet=SINGLE_PACKET,
                                queue_num=g % 4,
                            )
                            pos0 += nch
                        for (b, dcol, S_b, pos_list) in sg["blocks"]:
                            if S_b == 0:
                                continue
                            oh = ohpool.tile([P, SB_MAX, P], bf16, tag="oh")
                            d0 = dcol - sg["dcol0"]
                            bc = dst_t[:, d0:d0 + S_b].unsqueeze(2) \
                                .broadcast_to([P, S_b, P])
                            nc.vector.tensor_tensor(
                                out=oh[:, :S_b, :], in0=iota_big_t[:, :S_b, :],
                                in1=bc, op=Alu.is_equal)
                            pagg = ppool.tile([P, P], f32, tag="agg", space="PSUM")
                            if l == 0:
                                pdeg = ppool.tile([P, 1], f32, tag="deg", space="PSUM")
                            for j, pos in enumerate(pos_list):
                                nc.tensor.matmul(
                                    pagg[:], lhsT=oh[:, j, :], rhs=xg[:, pos, :],
                                    start=(j == 0), stop=(j == S_b - 1),
                                )
                                if l == 0:
                                    nc.tensor.matmul(
                                        pdeg[:], lhsT=oh[:, j, :], rhs=ones_bf[:],
                                        start=(j == 0), stop=(j == S_b - 1),
                                    )
                            # ---- fused epilogue ----
                            if l == 0:
                                dp1 = wpool.tile([P, 1], f32, tag="dp1")
                                nc.vector.tensor_scalar(
                                    out=dp1[:], in0=pdeg[:],
                                    scalar1=1.0, scalar2=None, op0=Alu.add,
                                )
                                nc.vector.reciprocal(inv_t[:, b:b + 1], dp1[:])
                            xs_bf = wpool.tile([P, P], bf16, tag="xs")
                            nc.sync.dma_start(out=xs_bf[:], in_=ag_in[l][b * P:(b + 1) * P, :])
                            xs = wpool.tile([P, P], f32, tag="xsf")
                            nc.scalar.copy(out=xs[:], in_=xs_bf[:])
                            m0 = wpool.tile([P, P], f32, tag="m0")
                            nc.vector.tensor_tensor(out=m0[:], in0=pagg[:], in1=xs[:], op=Alu.add)
                            m1 = wpool.tile([P, P], f32, tag="m1")
                            nc.vector.tensor_scalar(
                                out=m1[:], in0=m0[:], scalar1=inv_t[:, b:b + 1],
                                scalar2=None, op0=Alu.mult,
                            )
                            pT = ppool.tile([P, P], f32, tag="mT", space="PSUM")
                            nc.tensor.transpose(pT[:], m1[:], ident[:])
                            mT = wpool.tile([P, P], f32, tag="mTs")
                            nc.scalar.copy(out=mT[:], in_=pT[:])
                            ph = ppool.tile([P, P], f32, tag="h", space="PSUM")
                            nc.tensor.matmul(ph[:], lhsT=mT[:], rhs=Wl_ts[l][:], start=True, stop=True)
                            hb = wpool.tile([P, P], f32, tag="hb")
                            nc.vector.tensor_tensor(out=hb[:], in0=ph[:], in1=bl_t[l][:], op=Alu.add)
                            hr = wpool.tile([P, P], f32, tag="hr")
                            mu_s = wpool.tile([P, 1], f32, tag="mus")
                            nc.scalar.activation(hr[:], hb[:], Act.Relu, accum_out=mu_s[:])
                            h2 = wpool.tile([P, P], f32, tag="h2")
                            s2 = wpool.tile([P, 1], f32, tag="s2")
                            nc.scalar.activation(h2[:], hr[:], Act.Square, accum_out=s2[:])
                            musq = wpool.tile([P, 1], f32, tag="musq")
                            nc.vector.tensor_scalar(
                                out=musq[:], in0=mu_s[:], scalar1=mu_s[:, 0:1],
                                scalar2=1.0 / (P * P), op0=Alu.mult, op1=Alu.mult,
                            )
                            var1 = wpool.tile([P, 1], f32, tag="var1")
                            nc.vector.tensor_scalar(
                                out=var1[:], in0=s2[:], scalar1=1.0 / P,
                                scalar2=None, op0=Alu.mult,
                            )
                            var2 = wpool.tile([P, 1], f32, tag="var2")
                            nc.vector.tensor_tensor(out=var2[:], in0=var1[:], in1=musq[:], op=Alu.subtract)
                            std_t = wpool.tile([P, 1], f32, tag="std")
                            nc.scalar.activation(std_t[:], var2[:], Act.Sqrt, bias=eps_t[:, 0:1])
                            rstd = wpool.tile([P, 1], f32, tag="rstd")
                            nc.vector.reciprocal(rstd[:], std_t[:])
                            mu_t = wpool.tile([P, 1], f32, tag="mu")
                            nc.vector.tensor_scalar(
                                out=mu_t[:], in0=mu_s[:], scalar1=1.0 / P,
                                scalar2=None, op0=Alu.mult,
                            )
                            if not last:
                                y_bf = wpool.tile([P, P], bf16, tag="xnext")
                                nc.vector.tensor_scalar(
                                    out=y_bf[:], in0=hr[:], scalar1=mu_t[:, 0:1],
                                    scalar2=rstd[:, 0:1], op0=Alu.subtract, op1=Alu.mult,
                                )
                                nc.sync.dma_start(out=ag_in[l + 1][b * P:(b + 1) * P, :], in_=y_bf[:])
                            else:
                                y_f = wpool.tile([P, P], f32, tag="yf")
                                nc.vector.tensor_scalar(
                                    out=y_f[:], in0=hr[:], scalar1=mu_t[:, 0:1],
                                    scalar2=rstd[:, 0:1], op0=Alu.subtract, op1=Alu.mult,
                                )
                                pyT = ppool.tile([P, P], f32, tag="mT", space="PSUM")
                                nc.tensor.transpose(pyT[:], y_f[:], ident[:])
                                yT = wpool.tile([P, P], f32, tag="mTs")
                                nc.scalar.copy(out=yT[:], in_=pyT[:])
                                po = ppool.tile([P, OUT], f32, tag="h", space="PSUM")
                                nc.tensor.matmul(po[:], lhsT=yT[:], rhs=Wout_t[:], start=True, stop=True)
                                ob = wpool.tile([P, OUT], f32, tag="ob")
                                nc.vector.tensor_tensor(out=ob[:], in0=po[:], in1=bout_t[:], op=Alu.add)
                                nc.sync.dma_start(out=out_own[b * P:(b + 1) * P, :], in_=ob[:])
                    if not last:
                        nc.gpsimd.collective_compute(
                            "AllGather", Alu.bypass, replica_groups=rg,
                            ins=[ag_in[l + 1][:].opt()], outs=[x_full[l + 1][:].opt()],
                        )
    nc.compile()
    return nc


# --------------------------------------------------------------------------
# host driver
# --------------------------------------------------------------------------
def _run(nc_prog, in_maps):
    import jax
    import numpy as np
    from jax.sharding import Mesh, PartitionSpec, NamedSharding
    from jax.experimental.shard_map import shard_map
    import concourse.mybir as mybir
    from concourse.bass2jax import _bass_exec_p, install_neuronx_cc_hook, partition_id_tensor

    install_neuronx_cc_hook()
    nc = nc_prog
    partition_name = nc.partition_id_tensor.name if nc.partition_id_tensor else None
    in_names, out_names, out_avals, zero_outs = [], [], [], []
    for alloc in nc.m.functions[0].allocations:
        if not isinstance(alloc, mybir.MemoryLocationSet):
            continue
        name = alloc.memorylocations[0].name
        if alloc.kind == "ExternalInput":
            if name != partition_name:
                in_names.append(name)
        elif alloc.kind == "ExternalOutput":
            out_names.append(name)
            shape = tuple(alloc.tensor_shape)
            dtype = mybir.dt.np(alloc.dtype)
            out_avals.append(jax.core.ShapedArray(shape, dtype))
            zero_outs.append(np.zeros(shape, dtype))
    n_params = len(in_names)
    all_in = list(in_names) + list(out_names)
    if partition_name is not None:
        all_in.append(partition_name)

    def _body(*args):
        operands = list(args)
        if partition_name is not None:
            operands.append(partition_id_tensor())
        outs = _bass_exec_p.bind(
            *operands,
            out_avals=tuple(out_avals),
            in_names=tuple(all_in),
            out_names=tuple(out_names),
            lowering_input_output_aliases=(),
            sim_require_finite=False,
            sim_require_nnan=False,
            nc=nc,
        )
        return tuple(outs)

    devices = jax.devices()[:NC]
    mesh = Mesh(np.asarray(devices), ("core",))
    in_specs = (PartitionSpec("core"),) * (n_params + len(out_names))
    out_specs = (PartitionSpec("core"),) * len(out_names)
    fn = jax.jit(
        shard_map(_body, mesh=mesh, in_specs=in_specs, out_specs=out_specs,
                  check_rep=False),
        keep_unused=True,
    )
    concat_in = [
        np.concatenate([np.asarray(in_maps[c][k]) for c in range(NC)], axis=0)
        for k in in_names
    ]
    concat_zero = [np.zeros((NC * z.shape[0], *z.shape[1:]), z.dtype) for z in zero_outs]
    sharding = NamedSharding(mesh, PartitionSpec("core"))
    dev_in = [jax.device_put(a, sharding) for a in concat_in + concat_zero]
    outs = fn(*dev_in)
    jax.block_until_ready(outs)
    res = [
        {name: np.asarray(outs[i]).reshape(NC, *out_avals[i].shape)[c]
         for i, name in enumerate(out_names)}
        for c in range(NC)
    ]
    return res, (fn, dev_in, out_names, out_avals)


def _make_in_maps(inputs, N, SH, SHP, NB, LAYERS, OUT):
    import concourse.mybir as mybir
    bfnp = mybir.dt.np(mybir.dt.bfloat16)

    nodes = np.asarray(inputs["nodes"], np.float32)
    src = np.asarray(inputs["src"])
    dst = np.asarray(inputs["dst"])
    W_in = np.asarray(inputs["W_in"], np.float32)
    b_in = np.asarray(inputs["b_in"], np.float32)
    Ws = np.asarray(inputs["Ws"], np.float32)
    bs = np.asarray(inputs["bs"], np.float32)
    gammas = np.asarray(inputs["gammas"], np.float32)
    betas = np.asarray(inputs["betas"], np.float32)
    W_out = np.asarray(inputs["W_out"], np.float32)
    b_out = np.asarray(inputs["b_out"], np.float32)

    idx_flat, dst_flat, sgs = _prep_edges_uniform(src, dst, N, SH, SHP, NB)

    # fold LayerNorm gamma/beta into the following layer's weights
    Wl = np.zeros((LAYERS, P, P), np.float32)
    bl = np.zeros((LAYERS, P), np.float32)
    Wl[0] = Ws[0]
    bl[0] = bs[0]
    for l in range(1, LAYERS):
        Wl[l] = gammas[l - 1][:, None] * Ws[l]
        bl[l] = betas[l - 1] @ Ws[l] + bs[l]
    Wout = gammas[LAYERS - 1][:, None] * W_out
    bout = betas[LAYERS - 1] @ W_out + b_out

    SB_MAX = max(S_b for sg in sgs for (_b, _d, S_b, _p) in sg["blocks"])
    iota_big = np.tile(np.arange(P, dtype=np.float32), (P, SB_MAX, 1))

    b_in_bc = np.tile(b_in, (P, 1)).astype(np.float32)
    bl_bc = np.stack([np.tile(bl[l], (P, 1)) for l in range(LAYERS)])
    bout_bc = np.tile(bout, (P, 1)).astype(np.float32)

    in_maps = []
    for c in range(NC):
        nsh = np.zeros((SHP, P), np.float32)
        nsh[:SH] = nodes[c * SH:(c + 1) * SH]
        in_maps.append({
            "nodes_own": nsh,
            "idxs": idx_flat[c],
            "dsts": dst_flat[c].astype(bfnp),
            "W_in": W_in,
            "Wl": Wl,
            "Wout": Wout,
            "b_in_bc": b_in_bc,
            "bl_bc": bl_bc,
            "bout_bc": bout_bc,
            "iota_big": iota_big.astype(bfnp),
        })
    return in_maps, sgs


def kernel(**inputs):
    nodes = np.asarray(inputs["nodes"])
    N = nodes.shape[0]
    LAYERS = np.asarray(inputs["Ws"]).shape[0]
    OUT = np.asarray(inputs["W_out"]).shape[1]
    assert N % NC == 0
    SH = N // NC
    SHP = (SH + P - 1) // P * P
    NB = SHP // P
    assert SHP <= 32767, "int16 gather index limit"

    in_maps, sgs = _make_in_maps(inputs, N, SH, SHP, NB, LAYERS, OUT)
    ICOLS = in_maps[0]["idxs"].shape[1]
    DCOLS = in_maps[0]["dsts"].shape[1]

    key = (NB, SHP, ICOLS, DCOLS, LAYERS, OUT)
    if key not in _CACHE:
        _CACHE[key] = _build_program(NB, SHP, sgs, ICOLS, DCOLS, LAYERS, OUT)
    nc_prog = _CACHE[key]

    res, exec_info = _run(nc_prog, in_maps)
    global _LAST_EXEC
    _LAST_EXEC = exec_info
    out = np.concatenate([res[c]["out_own"][:SH] for c in range(NC)], axis=0)
    return out.astype(np.float32)
